# revision 28
# baseline (speedup 1.0000x reference)
"""
EntityEmbedding masked-mean kernel for Trainium2 (8 NeuronCores, SPMD).

Computes, per batch row b:
    mask  = (segment_ids[b] == 1)
    mask2 = mask with the FIRST set position cleared
    out[b] = sum_s mask2[s] * x[b, s, :] / sum(mask2)

Sharding: data-parallel over batch. B=32 rows -> 4 rows per core on 8 cores.

Device algorithm (per row):
  - seg row (f32 0/1 values) is DMA'd as [S/128, 128] and block-transposed on
    VectorE to the "chunk layout" maskT[p, c] = m[128c + p].
  - First set position via iota+min: val = iota + (1-mask)*65536, then a
    free-dim min (DVE) and a cross-partition min (GpSimd) give first_idx;
    mask2 = maskT * (iota != first_idx).
  - The 16 MiB x row streams through SBUF in 4 MiB strided DMAs laid out
    [128p, 8g, 1024h] (s = 1024t + 128g + p), and VectorE folds each
    [128, 1024] chunk into a per-lane accumulator with one fused
    scalar_tensor_tensor op: acc = x*mask_col + acc.
  - A ones-vector PE matmul reduces the 128 lanes, and the result is scaled
    by reciprocal(count-1).

All PE matmuls are structured to carry at most one semaphore wait (walrus's
fp32/transpose matmul lowering has a single sync-wait slot).
"""

import sys

import numpy as np

for _p in ("/opt/trn_rl_repo",):
    if _p not in sys.path:
        sys.path.insert(0, _p)

B, S, H = 32, 4096, 1024
NCORES = 8
R = B // NCORES  # batch rows per core
G = 8  # 128-row s-chunks per supertile
P = 128
BIG = 65536.0  # exact in fp32; dwarfs every iota value

_PROGRAM_CACHE = {}


def build_program(rows=R, s=S, h=H):
    import concourse.bacc as bacc
    import concourse.tile as tile
    from concourse import mybir

    fp32 = mybir.dt.float32
    Alu = mybir.AluOpType

    nchunk = s // P  # number of 128-row s-chunks
    g = min(G, nchunk)
    ntile = nchunk // g  # supertiles per row
    assert ntile * g == nchunk
    assert nchunk % 32 == 0, "block transpose needs 32-multiple chunk count"

    nc = bacc.Bacc("TRN2", debug=False, target_bir_lowering=False)

    x = nc.dram_tensor("x", [rows, s, h], fp32, kind="ExternalInput").ap()
    seg = nc.dram_tensor("seg", [rows, s], fp32, kind="ExternalInput").ap()
    out = nc.dram_tensor("out", [rows, h], fp32, kind="ExternalOutput").ap()

    # Lane layout: column c = g*t + k holds s = g*P*t + g*p + k, i.e. each
    # partition reads a contiguous 32 KiB run of each supertile (best DMA).
    iot_np = np.zeros((P, nchunk), np.float32)
    for t in range(ntile):
        for k in range(g):
            iot_np[:, g * t + k] = g * P * t + g * np.arange(P) + k
    iot_d = nc.inline_tensor(iot_np, "iot")
    iotb_d = nc.inline_tensor(iot_np + np.float32(BIG), "iotb")
    ones_d = nc.inline_tensor(np.ones((P, 1), np.float32), "ones")

    from contextlib import ExitStack

    with tile.TileContext(nc) as tc, ExitStack() as ctx:
        consts = ctx.enter_context(tc.tile_pool(name="consts", bufs=1))
        mrow = ctx.enter_context(tc.tile_pool(name="mrow", bufs=rows + 1))
        accp = ctx.enter_context(tc.tile_pool(name="accp", bufs=rows))
        xpool = ctx.enter_context(tc.tile_pool(name="xpool", bufs=5))
        resp = ctx.enter_context(tc.tile_pool(name="resp", bufs=2))
        psum = ctx.enter_context(tc.tile_pool(name="psum", bufs=2, space="PSUM"))

        iot_sb = consts.tile([P, nchunk], fp32)
        nc.sync.dma_start(out=iot_sb, in_=iot_d.ap())
        iotb_sb = consts.tile([P, nchunk], fp32)
        nc.sync.dma_start(out=iotb_sb, in_=iotb_d.ap())
        ones_sb = consts.tile([P, 1], fp32)
        nc.sync.dma_start(out=ones_sb, in_=ones_d.ap())

        # PE warm-up on the ones constant: after this, no later matmul needs
        # to wait on the constant DMA (transitive wait elision).
        dummy_ps = psum.tile([1, 1], fp32, tag="dummy_ps")
        nc.tensor.matmul(
            dummy_ps, lhsT=ones_sb[0:1, :], rhs=ones_sb[0:1, :], start=True, stop=True
        )

        masks = []
        recips = []
        # ---- mask prep for all rows up front (tiny; overlaps the stream) ----
        for r in range(rows):
            # maskT[p, g*t+k] = m[g*P*t + g*p + k] -- matches the x lane layout
            maskT = mrow.tile([P, nchunk], fp32, tag="maskT")
            nc.sync.dma_start(
                out=maskT.rearrange("p (t k) -> p t k", t=ntile),
                in_=seg[r].rearrange("(t p k) -> p t k", t=ntile, p=P),
            )

            # val = iota + (1-mask)*BIG
            val = mrow.tile([P, nchunk], fp32, tag="val")
            nc.vector.scalar_tensor_tensor(
                out=val,
                in0=maskT,
                scalar=-BIG,
                in1=iotb_sb,
                op0=Alu.mult,
                op1=Alu.add,
            )

            # Gather per-partition min(val) / count into column 0 of a tile,
            # block-transpose, and reduce along row 0 (partition 0).
            gmin = mrow.tile([P, 32], fp32, tag="gmin")
            nc.vector.memset(gmin, 0.0)
            nc.vector.tensor_reduce(
                out=gmin[:, 0:1], in_=val, axis=mybir.AxisListType.X, op=Alu.min
            )
            gcnt = mrow.tile([P, 32], fp32, tag="gcnt")
            nc.vector.memset(gcnt, 0.0)
            nc.vector.tensor_reduce(
                out=gcnt[:, 0:1], in_=maskT, axis=mybir.AxisListType.X, op=Alu.add
            )
            gminT = mrow.tile([32, P], fp32, tag="gminT")
            gcntT = mrow.tile([32, P], fp32, tag="gcntT")
            for jb in range(P // 32):
                nc.vector.transpose(
                    out=gminT[:, 32 * jb : 32 * jb + 32],
                    in_=gmin[32 * jb : 32 * jb + 32, :],
                )
                nc.vector.transpose(
                    out=gcntT[:, 32 * jb : 32 * jb + 32],
                    in_=gcnt[32 * jb : 32 * jb + 32, :],
                )
            first = mrow.tile([1, 1], fp32, tag="first")
            nc.vector.tensor_reduce(
                out=first, in_=gminT[0:1, :], axis=mybir.AxisListType.X, op=Alu.min
            )
            cnt = mrow.tile([1, 1], fp32, tag="cnt")
            nc.vector.tensor_reduce(
                out=cnt, in_=gcntT[0:1, :], axis=mybir.AxisListType.X, op=Alu.add
            )
            nc.vector.tensor_scalar_add(out=cnt, in0=cnt, scalar1=-1.0)
            recip_sb = mrow.tile([1, 1], fp32, tag="recip_sb")
            nc.vector.reciprocal(out=recip_sb, in_=cnt)

            # broadcast first across all 128 partitions via transpose
            fbc = mrow.tile([32, P], fp32, tag="fbc")
            nc.vector.memset(fbc, 0.0)
            nc.vector.tensor_scalar(
                out=fbc[0:1, :],
                in0=gminT[0:1, :],
                scalar1=0.0,
                scalar2=first,
                op0=Alu.mult,
                op1=Alu.add,
            )
            fbcT = mrow.tile([P, 32], fp32, tag="fbcT")
            for jb in range(P // 32):
                nc.vector.transpose(
                    out=fbcT[32 * jb : 32 * jb + 32, :],
                    in_=fbc[:, 32 * jb : 32 * jb + 32],
                )

            # mask2 = maskT * (iota != first)
            mask2 = mrow.tile([P, nchunk], fp32, tag="mask2")
            nc.vector.scalar_tensor_tensor(
                out=mask2,
                in0=iot_sb,
                scalar=fbcT[:, 0:1],
                in1=maskT,
                op0=Alu.not_equal,
                op1=Alu.mult,
            )

            masks.append(mask2)
            recips.append(recip_sb)

        # ---- main stream ----
        # Each 128-row s-chunk is folded in by either VectorE (fused
        # multiply-accumulate into an SBUF accumulator) or TensorE (mask
        # column as a [128,1] stationary vector, accumulating in PSUM), so
        # neither engine has to keep up with the full HBM stream alone.
        # DVE takes ~18/32 chunks, PE ~14/32 (balanced for ~110us each/core).
        n_pe = max(1, round(nchunk * 13 / 32))
        dve_chunks = {
            c
            for c in range(nchunk)
            if (c * n_pe) // nchunk == ((c + 1) * n_pe) // nchunk
        }
        for r in range(rows):
            mask2 = masks[r]
            acc = accp.tile([P, h], fp32, tag="acc")
            rs_ps = [
                psum.tile([1, 512], fp32, tag=f"rs_ps{half}", name=f"rs_ps{half}_{r}")
                for half in range(h // 512)
            ]
            pe_first = True
            dve_first = True
            for t in range(ntile):
                xt = xpool.tile([P, g, h], fp32, tag="xt")
                src_ap = x[r, t * g * P : (t + 1) * g * P, :].rearrange(
                    "(p k) h -> p k h", p=P
                )
                if r == rows - 1 and t == ntile - 1:
                    # final tile: two halves so the drain starts sooner
                    gh = g // 2
                    nc.sync.dma_start(out=xt[:, :gh, :], in_=src_ap[:, :gh, :])
                    nc.sync.dma_start(out=xt[:, gh:, :], in_=src_ap[:, gh:, :])
                else:
                    nc.sync.dma_start(out=xt, in_=src_ap)
                for j in range(g):
                    c = t * g + j
                    if c in dve_chunks:
                        if dve_first:
                            nc.vector.tensor_scalar_mul(
                                out=acc, in0=xt[:, j, :], scalar1=mask2[:, c : c + 1]
                            )
                            dve_first = False
                        else:
                            nc.vector.scalar_tensor_tensor(
                                out=acc,
                                in0=xt[:, j, :],
                                scalar=mask2[:, c : c + 1],
                                in1=acc,
                                op0=Alu.mult,
                                op1=Alu.add,
                            )
                    else:
                        for half in range(h // 512):
                            nc.tensor.matmul(
                                rs_ps[half],
                                lhsT=mask2[:, c : c + 1],
                                rhs=xt[:, j, half * 512 : (half + 1) * 512],
                                start=pe_first,
                                stop=False,
                            )
                        pe_first = False

            # fold the DVE accumulator into the same PSUM tiles and scale
            res_sb = resp.tile([1, h], fp32, tag="res_sb")
            for half in range(h // 512):
                nc.tensor.matmul(
                    rs_ps[half],
                    lhsT=ones_sb,
                    rhs=acc[:, half * 512 : (half + 1) * 512],
                    start=False,
                    stop=True,
                )
                nc.scalar.activation(
                    out=res_sb[:, half * 512 : (half + 1) * 512],
                    in_=rs_ps[half],
                    func=mybir.ActivationFunctionType.Copy,
                    scale=recips[r],
                )
            nc.scalar.dma_start(out=out[r : r + 1, :], in_=res_sb)

    nc.compile()
    return nc


def get_program():
    key = (R, S, H)
    if key not in _PROGRAM_CACHE:
        _PROGRAM_CACHE[key] = build_program()
    return _PROGRAM_CACHE[key]


def run_on_hw(x_full, seg_full, trace=False, **kw):
    from concourse.bass_utils import run_bass_kernel_spmd

    nc = get_program()
    in_maps = [
        {
            "x": np.ascontiguousarray(x_full[i * R : (i + 1) * R]),
            "seg": np.ascontiguousarray(seg_full[i * R : (i + 1) * R]),
        }
        for i in range(NCORES)
    ]
    res = run_bass_kernel_spmd(nc, in_maps, list(range(NCORES)), trace=trace, **kw)
    outs = np.concatenate([res.results[i]["out"] for i in range(NCORES)], axis=0)
    return outs, res


def kernel(sequence_output, segment_ids):
    x_full = np.asarray(sequence_output, dtype=np.float32)
    seg_full = np.asarray(segment_ids).astype(np.float32)
    outs, _ = run_on_hw(x_full, seg_full)
    return outs[:, None, :].astype(np.float32)


# revision 29
# speedup vs baseline: 1.1766x; 1.1766x over previous
"""
EntityEmbedding masked-mean kernel for Trainium2 (8 NeuronCores, SPMD).

Computes, per batch row b:
    mask  = (segment_ids[b] == 1)
    mask2 = mask with the FIRST set position cleared
    out[b] = sum_s mask2[s] * x[b, s, :] / sum(mask2)

Sharding: data-parallel over batch. B=32 rows -> 4 rows per core on 8 cores.

Device algorithm (per row):
  - seg row (f32 0/1 values) is DMA'd as [S/128, 128] and block-transposed on
    VectorE to the "chunk layout" maskT[p, c] = m[128c + p].
  - First set position via iota+min: val = iota + (1-mask)*65536, then a
    free-dim min (DVE) and a cross-partition min (GpSimd) give first_idx;
    mask2 = maskT * (iota != first_idx).
  - The 16 MiB x row streams through SBUF in 4 MiB strided DMAs laid out
    [128p, 8g, 1024h] (s = 1024t + 128g + p), and VectorE folds each
    [128, 1024] chunk into a per-lane accumulator with one fused
    scalar_tensor_tensor op: acc = x*mask_col + acc.
  - A ones-vector PE matmul reduces the 128 lanes, and the result is scaled
    by reciprocal(count-1).

All PE matmuls are structured to carry at most one semaphore wait (walrus's
fp32/transpose matmul lowering has a single sync-wait slot).
"""

import sys

import numpy as np

for _p in ("/opt/trn_rl_repo",):
    if _p not in sys.path:
        sys.path.insert(0, _p)

B, S, H = 32, 4096, 1024
NCORES = 8
R = B // NCORES  # batch rows per core
G = 8  # 128-row s-chunks per supertile
P = 128
BIG = 65536.0  # exact in fp32; dwarfs every iota value

_PROGRAM_CACHE = {}


def build_program(rows=R, s=S, h=H):
    import concourse.bacc as bacc
    import concourse.tile as tile
    from concourse import mybir

    fp32 = mybir.dt.float32
    Alu = mybir.AluOpType

    nchunk = s // P  # number of 128-row s-chunks
    g = min(G, nchunk)
    ntile = nchunk // g  # supertiles per row
    assert ntile * g == nchunk
    assert nchunk % 32 == 0, "block transpose needs 32-multiple chunk count"

    nc = bacc.Bacc("TRN2", debug=False, target_bir_lowering=False)

    x = nc.dram_tensor("x", [rows, s, h], fp32, kind="ExternalInput").ap()
    seg = nc.dram_tensor("seg", [rows, s], fp32, kind="ExternalInput").ap()
    out = nc.dram_tensor("out", [rows, h], fp32, kind="ExternalOutput").ap()

    # Lane layout: column c = g*t + k holds s = g*P*t + g*p + k, i.e. each
    # partition reads a contiguous 32 KiB run of each supertile (best DMA).
    iot_np = np.zeros((P, nchunk), np.float32)
    for t in range(ntile):
        for k in range(g):
            iot_np[:, g * t + k] = g * P * t + g * np.arange(P) + k
    iot_d = nc.inline_tensor(iot_np, "iot")
    iotb_d = nc.inline_tensor(iot_np + np.float32(BIG), "iotb")
    ones_d = nc.inline_tensor(np.ones((P, 1), np.float32), "ones")

    from contextlib import ExitStack

    with tile.TileContext(nc) as tc, ExitStack() as ctx:
        consts = ctx.enter_context(tc.tile_pool(name="consts", bufs=1))
        mrow = ctx.enter_context(tc.tile_pool(name="mrow", bufs=rows + 1))
        accp = ctx.enter_context(tc.tile_pool(name="accp", bufs=rows))
        xpool = ctx.enter_context(tc.tile_pool(name="xpool", bufs=5))
        resp = ctx.enter_context(tc.tile_pool(name="resp", bufs=2))
        psum = ctx.enter_context(tc.tile_pool(name="psum", bufs=2, space="PSUM"))

        iot_sb = consts.tile([P, nchunk], fp32)
        nc.sync.dma_start(out=iot_sb, in_=iot_d.ap())
        iotb_sb = consts.tile([P, nchunk], fp32)
        nc.sync.dma_start(out=iotb_sb, in_=iotb_d.ap())
        ones_sb = consts.tile([P, 1], fp32)
        nc.sync.dma_start(out=ones_sb, in_=ones_d.ap())

        # PE warm-up on the ones constant: after this, no later matmul needs
        # to wait on the constant DMA (transitive wait elision).
        dummy_ps = psum.tile([1, 1], fp32, tag="dummy_ps")
        nc.tensor.matmul(
            dummy_ps, lhsT=ones_sb[0:1, :], rhs=ones_sb[0:1, :], start=True, stop=True
        )

        masks = []
        recips = []
        # ---- mask prep for all rows up front (tiny; overlaps the stream) ----
        for r in range(rows):
            # maskT[p, g*t+k] = m[g*P*t + g*p + k] -- matches the x lane layout
            maskT = mrow.tile([P, nchunk], fp32, tag="maskT")
            nc.sync.dma_start(
                out=maskT.rearrange("p (t k) -> p t k", t=ntile),
                in_=seg[r].rearrange("(t p k) -> p t k", t=ntile, p=P),
            )

            # val = iota + (1-mask)*BIG
            val = mrow.tile([P, nchunk], fp32, tag="val")
            nc.vector.scalar_tensor_tensor(
                out=val,
                in0=maskT,
                scalar=-BIG,
                in1=iotb_sb,
                op0=Alu.mult,
                op1=Alu.add,
            )

            # Gather per-partition min(val) / count into column 0 of a tile,
            # block-transpose, and reduce along row 0 (partition 0).
            gmin = mrow.tile([P, 32], fp32, tag="gmin")
            nc.vector.memset(gmin, 0.0)
            nc.vector.tensor_reduce(
                out=gmin[:, 0:1], in_=val, axis=mybir.AxisListType.X, op=Alu.min
            )
            gcnt = mrow.tile([P, 32], fp32, tag="gcnt")
            nc.vector.memset(gcnt, 0.0)
            nc.vector.tensor_reduce(
                out=gcnt[:, 0:1], in_=maskT, axis=mybir.AxisListType.X, op=Alu.add
            )
            gminT = mrow.tile([32, P], fp32, tag="gminT")
            gcntT = mrow.tile([32, P], fp32, tag="gcntT")
            for jb in range(P // 32):
                nc.vector.transpose(
                    out=gminT[:, 32 * jb : 32 * jb + 32],
                    in_=gmin[32 * jb : 32 * jb + 32, :],
                )
                nc.vector.transpose(
                    out=gcntT[:, 32 * jb : 32 * jb + 32],
                    in_=gcnt[32 * jb : 32 * jb + 32, :],
                )
            first = mrow.tile([1, 1], fp32, tag="first")
            nc.vector.tensor_reduce(
                out=first, in_=gminT[0:1, :], axis=mybir.AxisListType.X, op=Alu.min
            )
            cnt = mrow.tile([1, 1], fp32, tag="cnt")
            nc.vector.tensor_reduce(
                out=cnt, in_=gcntT[0:1, :], axis=mybir.AxisListType.X, op=Alu.add
            )
            nc.vector.tensor_scalar_add(out=cnt, in0=cnt, scalar1=-1.0)
            recip_sb = mrow.tile([1, 1], fp32, tag="recip_sb")
            nc.vector.reciprocal(out=recip_sb, in_=cnt)

            # broadcast first across all 128 partitions via transpose
            fbc = mrow.tile([32, P], fp32, tag="fbc")
            nc.vector.memset(fbc, 0.0)
            nc.vector.tensor_scalar(
                out=fbc[0:1, :],
                in0=gminT[0:1, :],
                scalar1=0.0,
                scalar2=first,
                op0=Alu.mult,
                op1=Alu.add,
            )
            fbcT = mrow.tile([P, 32], fp32, tag="fbcT")
            for jb in range(P // 32):
                nc.vector.transpose(
                    out=fbcT[32 * jb : 32 * jb + 32, :],
                    in_=fbc[:, 32 * jb : 32 * jb + 32],
                )

            # mask2 = maskT * (iota != first)
            mask2 = mrow.tile([P, nchunk], fp32, tag="mask2")
            nc.vector.scalar_tensor_tensor(
                out=mask2,
                in0=iot_sb,
                scalar=fbcT[:, 0:1],
                in1=maskT,
                op0=Alu.not_equal,
                op1=Alu.mult,
            )

            masks.append(mask2)
            recips.append(recip_sb)

        # ---- main stream ----
        # Each 128-row s-chunk is folded in by either VectorE (fused
        # multiply-accumulate into an SBUF accumulator) or TensorE (mask
        # column as a [128,1] stationary vector, accumulating in PSUM), so
        # neither engine has to keep up with the full HBM stream alone.
        # DVE takes ~18/32 chunks, PE ~14/32 (balanced for ~110us each/core).
        n_pe = max(1, round(nchunk * 13 / 32))
        dve_chunks = {
            c
            for c in range(nchunk)
            if (c * n_pe) // nchunk == ((c + 1) * n_pe) // nchunk
        }
        for r in range(rows):
            mask2 = masks[r]
            acc = accp.tile([P, h], fp32, tag="acc")
            rs_ps = [
                psum.tile([1, 512], fp32, tag=f"rs_ps{half}", name=f"rs_ps{half}_{r}")
                for half in range(h // 512)
            ]
            pe_first = True
            dve_first = True
            for t in range(ntile):
                xt = xpool.tile([P, g, h], fp32, tag="xt")
                src_ap = x[r, t * g * P : (t + 1) * g * P, :].rearrange(
                    "(p k) h -> p k h", p=P
                )
                if r == rows - 1 and t == ntile - 1:
                    # final tile: two halves so the drain starts sooner
                    gh = g // 2
                    nc.sync.dma_start(out=xt[:, :gh, :], in_=src_ap[:, :gh, :])
                    nc.sync.dma_start(out=xt[:, gh:, :], in_=src_ap[:, gh:, :])
                else:
                    nc.sync.dma_start(out=xt, in_=src_ap)
                for j in range(g):
                    c = t * g + j
                    if c in dve_chunks:
                        if dve_first:
                            nc.vector.tensor_scalar_mul(
                                out=acc, in0=xt[:, j, :], scalar1=mask2[:, c : c + 1]
                            )
                            dve_first = False
                        else:
                            nc.vector.scalar_tensor_tensor(
                                out=acc,
                                in0=xt[:, j, :],
                                scalar=mask2[:, c : c + 1],
                                in1=acc,
                                op0=Alu.mult,
                                op1=Alu.add,
                            )
                    else:
                        for half in range(h // 512):
                            nc.tensor.matmul(
                                rs_ps[half],
                                lhsT=mask2[:, c : c + 1],
                                rhs=xt[:, j, half * 512 : (half + 1) * 512],
                                start=pe_first,
                                stop=False,
                            )
                        pe_first = False

            # fold the DVE accumulator into the same PSUM tiles and scale
            res_sb = resp.tile([1, h], fp32, tag="res_sb")
            for half in range(h // 512):
                nc.tensor.matmul(
                    rs_ps[half],
                    lhsT=ones_sb,
                    rhs=acc[:, half * 512 : (half + 1) * 512],
                    start=False,
                    stop=True,
                )
                nc.scalar.activation(
                    out=res_sb[:, half * 512 : (half + 1) * 512],
                    in_=rs_ps[half],
                    func=mybir.ActivationFunctionType.Copy,
                    scale=recips[r],
                )
            nc.sync.dma_start(out=out[r : r + 1, :], in_=res_sb)

    nc.compile()
    return nc


def get_program():
    key = (R, S, H)
    if key not in _PROGRAM_CACHE:
        _PROGRAM_CACHE[key] = build_program()
    return _PROGRAM_CACHE[key]


def run_on_hw(x_full, seg_full, trace=False, **kw):
    from concourse.bass_utils import run_bass_kernel_spmd

    nc = get_program()
    in_maps = [
        {
            "x": np.ascontiguousarray(x_full[i * R : (i + 1) * R]),
            "seg": np.ascontiguousarray(seg_full[i * R : (i + 1) * R]),
        }
        for i in range(NCORES)
    ]
    res = run_bass_kernel_spmd(nc, in_maps, list(range(NCORES)), trace=trace, **kw)
    outs = np.concatenate([res.results[i]["out"] for i in range(NCORES)], axis=0)
    return outs, res


def kernel(sequence_output, segment_ids):
    x_full = np.asarray(sequence_output, dtype=np.float32)
    seg_full = np.asarray(segment_ids).astype(np.float32)
    outs, _ = run_on_hw(x_full, seg_full)
    return outs[:, None, :].astype(np.float32)
